# revision 21
# baseline (speedup 1.0000x reference)
"""Trainium2 Bass kernel for nn_BondLenConstrain.

Contract: kernel(**inputs) takes the FULL (unsharded) inputs of
reference.setup_inputs() and returns the full [64, 4, 2048, 2] float32
resiEnergy tensor.  Internally the work is data-parallel over the batch
axis across 8 NeuronCores (8 batches per core).

Approach
--------
Host (numpy, cheap indexing only):
  * scatter atoms into dense [nb, mc, mr] residue grids exactly like the
    reference (index_put with drop semantics), build the `todo` mask, and
    gather the tiny per-residue-type score tables into per-residue
    coefficient lanes.  Masked pairs get all-zero coefficients which makes
    the device formula return exactly 0 for them.
  * coords grid is shipped as G[nb, mc, mr+1, 9] (N, CA, C per residue; a
    zero row at index 0 serves as the left halo for residue 0).

Device (Bass/Tile, per core):
  Dense stencil over residue pairs.  Per pair (r-1, r):
      v2 = CA_r - N_r, v1 = C_{r-1} - N_r, v3 = CA_{r-1} - C_{r-1}
      d22,d11,d33,d12,d31 = dots
      bond       f1 = sqrt(d11)             = exp(0.5*ln d11)
      angles via  t = arctan(|c| / s),  s = sqrt(dxx*d11 - c^2)  (Lagrange)
      score_d = min(((f_d - mu_d)/ (sqrt(2)*std_d))^2, ln(q_d/(EPS*sqrt(pi))))
      e = s_w * sum_d score_d   (s_w = 1 - tanh(-weight))
  The arccos-with-clip of the reference is replaced by the arctan identity
  (theta = pi/2 - sign(c)*arctan(|c|/s)); the sign is applied to the
  (pi/2 - mu) coefficient so only squares are needed.  All normalisations
  go through exp/ln on the ScalarE tables (Rsqrt/Reciprocal are banned).

Layout per core: partition p = (batch, chain, block) = 8*4*4 = 128,
free dim = 512 residue pairs per partition (block of the chain).
"""

import os
import numpy as np

PAD = -999.0
PAD_I = -999
NB, MC, MR = 64, 4, 2048
NALT = 2
NCORES = 8
BPC = NB // NCORES            # batches per core
CH = int(os.environ.get("BLC_CHUNKS", "1"))  # pipeline chunks per core
KC = 4 * CH                   # blocks per (batch, chain)
R = MR // KC                  # residues (pairs) per partition
EPS = 1e-12
CL = 1.0 / (EPS * np.sqrt(np.pi))

_PROGRAM_CACHE = {}
LAST_RESULT = None            # BassKernelResults of the last run (for test.py)
TRACE = bool(int(os.environ.get("BLC_TRACE", "0")))


def _build_program():
    import concourse.bass as bass
    import concourse.tile as tile
    from concourse import bacc, mybir

    dt = mybir.dt.float32
    Alu = mybir.AluOpType
    Act = mybir.ActivationFunctionType

    nc = bacc.Bacc("TRN2", target_bir_lowering=False, debug=False)
    # extra const AP for the Sign bias (maps c == 0 to sign +1)
    _sgn_eps = 1e-35
    _ct = nc.alloc_sbuf_tensor("const-f32-sgneps", [128, 1], dt)
    nc.gpsimd.memset(_ct.ap(), _sgn_eps)
    nc.const_aps.aps[(dt, _sgn_eps)] = _ct.ap()
    nc.all_engine_barrier()

    G_t = nc.declare_dram_parameter("g", [BPC, MC, KC, (R + 1) * 9], dt,
                                    isOutput=False)
    P_t = nc.declare_dram_parameter("pr", [BPC, MC, KC, 9, R], dt, isOutput=False)
    O_t = nc.declare_dram_parameter("out", [BPC, MC, MR, NALT], dt, isOutput=True)

    bc = BPC // CH            # batches per chunk
    bufs = min(CH, 2)

    with tile.TileContext(nc) as tc:
        with (
            tc.tile_pool(name="px", bufs=bufs) as px,
            tc.tile_pool(name="pv", bufs=bufs) as pv,
            tc.tile_pool(name="pp", bufs=bufs) as pp,
            tc.tile_pool(name="ps", bufs=bufs) as ps,
        ):
            for c in range(CH):
                b0 = c * bc
                # ---- loads -------------------------------------------------
                X = px.tile([128, (R + 1) * 9], dt, tag="x")
                nc.sync.dma_start(X[:], G_t[b0:b0 + bc])
                P = pp.tile([128, 9 * R], dt, tag="p")
                nc.sync.dma_start(
                    P[:],
                    P_t[b0:b0 + bc].rearrange("b c k f l -> b c k (f l)"),
                )
                # Early DVE consumer of P: absorbs the P-DMA semaphore wait
                # into the DVE clock so later joins (AV, W) each carry only
                # one fresh wait (walrus allows a single embedded sync-wait).
                PA = pp.tile([128, R], dt, tag="pa")
                nc.vector.tensor_copy(PA[:], P[:, 0:R])

                # ---- stage A: difference vectors ---------------------------
                Xr = X[:].rearrange("p (l n) -> p l n", n=9)
                NN = Xr[:, 1:R + 1, 0:3]
                CAN = Xr[:, 1:R + 1, 3:6]
                CPV = Xr[:, 0:R, 6:9]
                CAP = Xr[:, 0:R, 3:6]
                V = pv.tile([128, R * 9], dt, tag="v")
                Vr = V[:].rearrange("p (l n) -> p l n", n=9)
                nc.vector.tensor_sub(Vr[:, :, 0:3], CAN, NN)   # v2
                nc.vector.tensor_sub(Vr[:, :, 3:6], CPV, NN)   # v1
                nc.vector.tensor_sub(Vr[:, :, 6:9], CAP, CPV)  # v3

                # ---- stage B: dot products ---------------------------------
                SQ = px.tile([128, R * 9], dt, tag="sq")
                nc.scalar.activation(SQ[:], V[:], Act.Square)
                SQr = SQ[:].rearrange("p (l v n) -> p l v n", v=3, n=3)
                D1 = ps.tile([128, 3 * R], dt, tag="d1")
                D1v = D1[:].rearrange("p (v l) -> p l v", v=3)
                nc.vector.tensor_add(D1v, SQr[:, :, :, 0], SQr[:, :, :, 1])
                nc.vector.tensor_add(D1v, D1v, SQr[:, :, :, 2])
                d22, d11, d33 = D1[:, 0:R], D1[:, R:2 * R], D1[:, 2 * R:3 * R]

                Vq = V[:].rearrange("p (l m n) -> p l m n", m=3, n=3)
                CP = pv.tile([128, 6 * R], dt, tag="cp")
                CPr = CP[:].rearrange("p (l m n) -> p l m n", m=2, n=3)
                nc.vector.tensor_mul(CPr, Vq[:, :, 1:3, :], Vq[:, :, 0:2, :])
                DC = ps.tile([128, 2 * R], dt, tag="dc")
                DCv = DC[:].rearrange("p (m l) -> p l m", m=2)
                nc.vector.tensor_add(DCv, CPr[:, :, :, 0], CPr[:, :, :, 1])
                nc.vector.tensor_add(DCv, DCv, CPr[:, :, :, 2])

                # ---- stage C: angles via exp/ln/arctan ---------------------
                M = ps.tile([128, 2 * R], dt, tag="m")
                nc.vector.tensor_mul(M[:, 0:R], d22, d11)
                nc.vector.tensor_mul(M[:, R:2 * R], d33, d11)
                PSQ = ps.tile([128, 2 * R], dt, tag="psq")
                nc.scalar.activation(PSQ[:], DC[:], Act.Square)
                S2 = ps.tile([128, 2 * R], dt, tag="s2")
                nc.vector.tensor_sub(S2[:], M[:], PSQ[:])
                SG = ps.tile([128, 2 * R], dt, tag="sg")
                nc.scalar.activation(SG[:], DC[:], Act.Sign, bias=1e-35)
                LNIN = ps.tile([128, 5 * R], dt, tag="lnin")
                nc.vector.tensor_scalar_max(LNIN[:, 0:2 * R], S2[:], 1e-30)
                # |c| = c * sign(c), clamped away from 0 for the Ln
                nc.vector.tensor_mul(LNIN[:, 2 * R:4 * R], DC[:], SG[:])
                nc.vector.tensor_scalar_max(
                    LNIN[:, 2 * R:4 * R], LNIN[:, 2 * R:4 * R], 1e-35)
                nc.vector.tensor_scalar_max(LNIN[:, 4 * R:5 * R], d11, 1e-30)
                LNO = pv.tile([128, 5 * R], dt, tag="cp")      # reuse CP slot
                nc.scalar.activation(LNO[:], LNIN[:], Act.Ln)
                # r = ln|c| - 0.5*ln(s2);  t = arctan(exp(r)) computed with
                # range reduction (ScalarE arctan domain is [-pi/2, pi/2]):
                #   t' = arctan(exp(-|r|)) in [0, pi/4]
                #   t  = pi/4 + sign(r)*(pi/4 - t')
                RT = ps.tile([128, 2 * R], dt, tag="rt")
                nc.vector.scalar_tensor_tensor(
                    RT[:], LNO[:, 0:2 * R], -0.5, LNO[:, 2 * R:4 * R],
                    op0=Alu.mult, op1=Alu.add)
                SR = ps.tile([128, 2 * R], dt, tag="sr")
                nc.scalar.activation(SR[:], RT[:], Act.Sign)
                ABSR = ps.tile([128, 2 * R], dt, tag="psq")    # reuse PSQ
                nc.vector.tensor_mul(ABSR[:], RT[:], SR[:])
                EN = ps.tile([128, 2 * R], dt, tag="s2")       # reuse S2
                nc.scalar.activation(EN[:], ABSR[:], Act.Exp, scale=-1.0)
                TP = ps.tile([128, 2 * R], dt, tag="tp")
                nc.scalar.activation(TP[:], EN[:], Act.Arctan)
                TB = ps.tile([128, 2 * R], dt, tag="tb")
                nc.vector.tensor_scalar(TB[:], TP[:], -1.0, np.pi / 4,
                                        op0=Alu.mult, op1=Alu.add)
                TC = ps.tile([128, 2 * R], dt, tag="tc")
                nc.vector.tensor_mul(TC[:], SR[:], TB[:])
                F1 = ps.tile([128, R], dt, tag="f1")
                nc.scalar.activation(F1[:], LNO[:, 4 * R:5 * R],
                                     Act.Exp, scale=0.5)

                # ---- stage D: scores ---------------------------------------
                AV = ps.tile([128, 2 * R], dt, tag="av")
                nc.vector.tensor_mul(AV[:], SG[:], P[:, R:3 * R])
                W = ps.tile([128, 3 * R], dt, tag="w")
                nc.vector.tensor_mul(W[:, 0:R], F1[:], P[:, 3 * R:4 * R])
                nc.vector.scalar_tensor_tensor(
                    W[:, R:3 * R], TC[:], np.pi / 4, P[:, 4 * R:6 * R],
                    op0=Alu.add, op1=Alu.mult)
                U = ps.tile([128, 3 * R], dt, tag="u")
                nc.vector.tensor_sub(U[:, 0:R], W[:, 0:R], PA[:])
                nc.vector.tensor_sub(U[:, R:3 * R], W[:, R:3 * R], AV[:])
                Z = ps.tile([128, 3 * R], dt, tag="w")         # reuse W? W read by U only
                nc.vector.tensor_mul(Z[:], U[:], U[:])
                ZC = ps.tile([128, 3 * R], dt, tag="u")        # hmm U still read by Z
                nc.vector.tensor_tensor(ZC[:], Z[:], P[:, 6 * R:9 * R], op=Alu.min)
                E = ps.tile([128, R], dt, tag="e")
                nc.vector.tensor_add(E[:], ZC[:, 0:R], ZC[:, R:2 * R])
                nc.vector.tensor_add(E[:], E[:], ZC[:, 2 * R:3 * R])
                O = ps.tile([128, 2 * R], dt, tag="o")
                Ov = O[:].rearrange("p (l a) -> p a l", a=2)
                nc.scalar.copy(Ov[:, 0, :], E[:])
                nc.scalar.copy(Ov[:, 1, :], E[:])
                nc.sync.dma_start(
                    O_t[b0:b0 + bc].rearrange(
                        "b c (k l) a -> b c k (l a)", k=KC),
                    O[:],
                )
    return nc


def _get_program():
    if "nc" not in _PROGRAM_CACHE:
        nc = _build_program()
        nc.finalize()   # Bacc: register allocation / DCE / wait legalization
        _PROGRAM_CACHE["nc"] = nc
    return _PROGRAM_CACHE["nc"]


def _host_prep(atom_description, coords, mean, std, weight):
    ad = np.asarray(atom_description)
    coords = np.asarray(coords, dtype=np.float32)
    b, ch, rs, rn, an = (ad[:, i] for i in range(5))
    valid = (b >= 0) & (b < NB) & (ch >= 0) & (ch < MC) & (rs >= 0) & (rs < MR)

    def scat3(mask):
        A = np.full((NB, MC, MR, 3), PAD, np.float32)
        m = mask & valid
        A[b[m], ch[m], rs[m]] = coords[m]
        return A

    Narr, CAarr, Carr = scat3(an == 0), scat3(an == 1), scat3(an == 2)
    seq = np.full((NB, MC, MR), PAD_I, np.int64)
    m = (an == 1) & valid
    seq[b[m], ch[m], rs[m]] = rn[m]

    todo = ((Narr[:, :, 1:, 0] != PAD) & (Carr[:, :, :-1, 0] != PAD)
            & (CAarr[:, :, 1:, 0] != PAD) & (CAarr[:, :, :-1, 0] != PAD)
            & (seq[:, :, 1:] != PAD_I) & (seq[:, :, :-1] != PAD_I))
    sidx = np.clip(np.where(todo, seq[:, :, 1:], 0), 0, 19)

    w0 = float(np.asarray(weight).reshape(-1)[0])
    s_w = 1.0 - np.tanh(-w0)
    sq = np.sqrt(s_w)
    mu = np.asarray(mean, np.float64)
    sd = np.asarray(std, np.float64)
    q = 1.0 / (sd * np.sqrt(2.0))
    tab = np.empty((20, 9))
    tab[:, 0] = mu[:, 0] * q[:, 0] * sq
    tab[:, 1] = (np.pi / 2 - mu[:, 1]) * q[:, 1] * sq
    tab[:, 2] = (mu[:, 2] - np.pi / 2) * q[:, 2] * sq
    tab[:, 3:6] = q * sq
    tab[:, 6:9] = s_w * np.maximum(np.log(CL * q), 0.0)
    tab = tab.astype(np.float32)

    params = np.zeros((NB, MC, MR, 9), np.float32)
    params[:, :, 1:, :] = tab[sidx] * todo[..., None].astype(np.float32)
    # blocked layout [NB, MC, KC, 9, R] (coefficient-major per block)
    pblk = np.ascontiguousarray(
        params.reshape(NB, MC, KC, R, 9).transpose(0, 1, 2, 4, 3))

    G = np.zeros((NB, MC, MR + 1, 9), np.float32)
    G[:, :, 1:, 0:3] = Narr
    G[:, :, 1:, 3:6] = CAarr
    G[:, :, 1:, 6:9] = Carr
    # blocked-with-halo: GB[b, c, k, :] = G[b, c, k*R : k*R+R+1, :].ravel()
    GB = np.empty((NB, MC, KC, (R + 1) * 9), np.float32)
    for k in range(KC):
        GB[:, :, k, :] = G[:, :, k * R:k * R + R + 1, :].reshape(NB, MC, -1)
    return GB, pblk


def _install_ntff_hook():
    """The agent image's antenv lacks axon_hooks; synthesize it so
    trace=True can reach the terminal's NRT profiler (dev-only path)."""
    import sys, types
    if "antenv.axon_hooks" in sys.modules:
        return True
    try:
        import antenv
        mod = types.ModuleType("antenv.axon_hooks")
        mod._hook = None

        def set_axon_ntff_profile_hook(h):
            mod._hook = h

        def get_axon_ntff_profile_hook():
            return mod._hook

        mod.set_axon_ntff_profile_hook = set_axon_ntff_profile_hook
        mod.get_axon_ntff_profile_hook = get_axon_ntff_profile_hook
        sys.modules["antenv.axon_hooks"] = mod
        antenv.axon_hooks = mod
        from trn_agent_boot.trn_boot import _ntff_profile_via_ctypes
        mod._hook = _ntff_profile_via_ctypes("/opt/axon/libaxon_pjrt.so")
        return True
    except Exception as e:  # pragma: no cover - profiling is best-effort
        print(f"ntff hook install failed: {e}")
        return False


def kernel(**inputs):
    global LAST_RESULT
    from concourse.bass_utils import run_bass_kernel_spmd
    if TRACE:
        _install_ntff_hook()

    G, pblk = _host_prep(
        inputs["atom_description"], inputs["coords"],
        inputs["mean"], inputs["std"], inputs["weight"])

    nc = _get_program()
    in_maps = [
        {"g": np.ascontiguousarray(G[i * BPC:(i + 1) * BPC]),
         "pr": np.ascontiguousarray(pblk[i * BPC:(i + 1) * BPC])}
        for i in range(NCORES)
    ]
    res = run_bass_kernel_spmd(nc, in_maps, list(range(NCORES)), trace=TRACE)
    LAST_RESULT = res
    out = np.concatenate([res.results[i]["out"] for i in range(NCORES)], axis=0)
    return np.ascontiguousarray(out.reshape(NB, MC, MR, NALT).astype(np.float32))


# revision 23
# speedup vs baseline: 1.1925x; 1.1925x over previous
"""Trainium2 Bass kernel for nn_BondLenConstrain.

Contract: kernel(**inputs) takes the FULL (unsharded) inputs of
reference.setup_inputs() and returns the full [64, 4, 2048, 2] float32
resiEnergy tensor.  Data-parallel over the batch axis across 8 NeuronCores
(8 batches per core).

Host (numpy, indexing only): scatter atoms into dense residue grids exactly
like the reference, build the `todo` mask, gather the tiny per-residue-type
tables into per-residue coefficient planes (masked pairs get all-zero
coefficients -> device formula returns exactly 0), transpose coords to a
plane-contiguous blocked layout, and broadcast the (identical) nalt lanes
of the output.

Device math per residue pair (r-1, r):
    v2 = CA_r - N_r, v1 = C_{r-1} - N_r, v3 = CA_{r-1} - C_{r-1}
    bond  f1 = sqrt(d11) = exp(0.5 ln d11)
    angle theta = pi/2 - sign(c) * arctan(|c|/s),  s = sqrt(dxx*d11 - c^2)
          arctan over [0,inf) via  t' = arctan(exp(-|ln(|c|/s)|)) in [0,pi/4]
          (ScalarE arctan domain is [-pi/2, pi/2])
    score_d = min(((f_d - mu_d) / (sqrt2 sigma_d))^2, ln(q_d/(EPS sqrt(pi))))
    e = s_w * sum_d score_d
All signs are folded into squared terms; normalisations go through exp/ln
(ScalarE Rsqrt/Reciprocal are disallowed).

Layout per core/chunk: partition p = (batch, chain, block); free dim =
plane-contiguous arrays of R residue pairs (unit-stride runs for the DVE).
Two chunks pipeline DMA/DVE/ACT/GPSIMD; all Ln/Exp activations are emitted
for both chunks before any Arctan so the ACT table set loads exactly twice.
"""

import os
import numpy as np

PAD = -999.0
PAD_I = -999
NB, MC, MR = 64, 4, 2048
NALT = 2
NCORES = 8
BPC = NB // NCORES            # batches per core
CH = int(os.environ.get("BLC_CHUNKS", "2"))  # pipeline chunks per core
KC = (4 * CH) // 1            # blocks per (batch, chain) across full chain
R = MR // KC                  # residues (pairs) per partition
EPS = 1e-12
CL = 1.0 / (EPS * np.sqrt(np.pi))

_PROGRAM_CACHE = {}
LAST_RESULT = None            # BassKernelResults of the last run (for test.py)
TRACE = bool(int(os.environ.get("BLC_TRACE", "0")))


def _build_program():
    import concourse.bass as bass
    import concourse.tile as tile
    from concourse import bacc, mybir

    dt = mybir.dt.float32
    Alu = mybir.AluOpType
    Act = mybir.ActivationFunctionType

    nc = bacc.Bacc("TRN2", target_bir_lowering=False, debug=False)
    # extra const AP for the Sign bias (maps c == 0 to sign +1)
    _sgn_eps = 1e-35
    _ct = nc.alloc_sbuf_tensor("const-f32-sgneps", [128, 1], dt)
    nc.gpsimd.memset(_ct.ap(), _sgn_eps)
    nc.const_aps.aps[(dt, _sgn_eps)] = _ct.ap()
    nc.all_engine_barrier()

    G_t = nc.declare_dram_parameter("g", [BPC, MC, KC, 9, R + 1], dt,
                                    isOutput=False)
    P_t = nc.declare_dram_parameter("pr", [BPC, MC, KC, 9, R], dt,
                                    isOutput=False)
    O_t = nc.declare_dram_parameter("out", [BPC, MC, MR], dt, isOutput=True)

    bc = BPC // CH            # batches per chunk
    bufs = min(CH, 2)
    S = R + 1                 # slots per coord plane

    with tile.TileContext(nc) as tc:
        with (
            tc.tile_pool(name="px", bufs=bufs) as px,
            tc.tile_pool(name="pp", bufs=bufs) as pp,
            tc.tile_pool(name="ps", bufs=bufs) as ps,
        ):
            state = []
            # ---------------- phase 1: loads, geometry, ln/exp -------------
            for c in range(CH):
                b0 = c * bc
                X = px.tile([128, 9 * S], dt, tag="x")
                nc.sync.dma_start(X[:], G_t[b0:b0 + bc])
                P = pp.tile([128, 9 * R], dt, tag="p")
                nc.sync.dma_start(P[:], P_t[b0:b0 + bc])

                def xpl(p, off):   # coord plane p, slot offset
                    return X[:, p * S + off: p * S + off + R]

                # difference vectors, plane-contiguous [v2|v1|v3] x (x,y,z)
                V = px.tile([128, 9 * R], dt, tag="v")
                Vv = V[:].rearrange("p (v c l) -> p v c l", v=3, c=3)
                Xv = X[:].rearrange("p (n l) -> p n l", n=9)
                # v2 = CA(l+1) - N(l+1)
                nc.vector.tensor_sub(Vv[:, 0], Xv[:, 3:6, 1:S], Xv[:, 0:3, 1:S])
                # v1 = C(l) - N(l+1)
                nc.vector.tensor_sub(Vv[:, 1], Xv[:, 6:9, 0:R], Xv[:, 0:3, 1:S])
                # v3 = CA(l) - C(l)
                nc.vector.tensor_sub(Vv[:, 2], Xv[:, 3:6, 0:R], Xv[:, 6:9, 0:R])

                SQ = px.tile([128, 9 * R], dt, tag="sq")
                nc.scalar.activation(SQ[:], V[:], Act.Square)
                SQv = SQ[:].rearrange("p (v c l) -> p v c l", v=3, c=3)
                # D1 = [d22 | d11 | d33]
                D1 = ps.tile([128, 3 * R], dt, tag="d1")
                D1v = D1[:].rearrange("p (v l) -> p v l", v=3)
                nc.vector.tensor_add(D1v, SQv[:, :, 0], SQv[:, :, 1])
                nc.vector.tensor_add(D1v, D1v, SQv[:, :, 2])
                # cross products [v1*v2 | v3*v1]
                CP = ps.tile([128, 6 * R], dt, tag="cp")
                nc.vector.tensor_mul(CP[:], V[:, 3 * R:9 * R], V[:, 0:6 * R])
                CPv = CP[:].rearrange("p (m c l) -> p m c l", m=2, c=3)
                DC = ps.tile([128, 2 * R], dt, tag="dc")
                DCv = DC[:].rearrange("p (m l) -> p m l", m=2)
                nc.vector.tensor_add(DCv, CPv[:, :, 0], CPv[:, :, 1])
                nc.vector.tensor_add(DCv, DCv, CPv[:, :, 2])

                M = ps.tile([128, 2 * R], dt, tag="m")
                nc.vector.tensor_mul(M[:, 0:R], D1[:, 0:R], D1[:, R:2 * R])
                nc.vector.tensor_mul(M[:, R:2 * R], D1[:, 2 * R:3 * R],
                                     D1[:, R:2 * R])
                PSQ = ps.tile([128, 2 * R], dt, tag="psq")
                nc.scalar.activation(PSQ[:], DC[:], Act.Square)
                SG = ps.tile([128, 2 * R], dt, tag="sg")
                nc.scalar.activation(SG[:], DC[:], Act.Sign, bias=1e-35)
                S2 = ps.tile([128, 2 * R], dt, tag="s2")
                nc.vector.tensor_sub(S2[:], M[:], PSQ[:])
                LNIN = ps.tile([128, 5 * R], dt, tag="lnin")
                nc.vector.tensor_scalar_max(LNIN[:, 0:2 * R], S2[:], 1e-30)
                nc.vector.tensor_mul(LNIN[:, 2 * R:4 * R], DC[:], SG[:])
                nc.vector.tensor_scalar_max(
                    LNIN[:, 2 * R:4 * R], LNIN[:, 2 * R:4 * R], 1e-35)
                nc.vector.tensor_scalar_max(LNIN[:, 4 * R:5 * R],
                                            D1[:, R:2 * R], 1e-30)
                LNO = ps.tile([128, 5 * R], dt, tag="lno")
                nc.scalar.activation(LNO[:], LNIN[:], Act.Ln)
                RT = ps.tile([128, 2 * R], dt, tag="rt")
                nc.vector.scalar_tensor_tensor(
                    RT[:], LNO[:, 0:2 * R], -0.5, LNO[:, 2 * R:4 * R],
                    op0=Alu.mult, op1=Alu.add)
                SR = ps.tile([128, 2 * R], dt, tag="sr")
                nc.scalar.activation(SR[:], RT[:], Act.Sign)
                ABSR = ps.tile([128, 2 * R], dt, tag="absr")
                nc.vector.tensor_mul(ABSR[:], RT[:], SR[:])
                EN = ps.tile([128, 2 * R], dt, tag="en")
                nc.scalar.activation(EN[:], ABSR[:], Act.Exp, scale=-1.0)
                F1 = ps.tile([128, R], dt, tag="f1")
                nc.scalar.activation(F1[:], LNO[:, 4 * R:5 * R],
                                     Act.Exp, scale=0.5)
                state.append((b0, P, EN, SR, SG, F1))

            # ---------------- phase 2: arctan + scoring --------------------
            for c in range(CH):
                b0, P, EN, SR, SG, F1 = state[c]
                TP = ps.tile([128, 2 * R], dt, tag="tp")
                nc.scalar.activation(TP[:], EN[:], Act.Arctan)
                TB = ps.tile([128, 2 * R], dt, tag="tb")
                nc.vector.tensor_scalar(TB[:], TP[:], -1.0, np.pi / 4,
                                        op0=Alu.mult, op1=Alu.add)
                TC = ps.tile([128, 2 * R], dt, tag="tc")
                nc.vector.tensor_mul(TC[:], SR[:], TB[:])
                AV = ps.tile([128, 2 * R], dt, tag="av")
                nc.vector.tensor_mul(AV[:], SG[:], P[:, R:3 * R])
                W = ps.tile([128, 3 * R], dt, tag="w")
                nc.vector.tensor_mul(W[:, 0:R], F1[:], P[:, 3 * R:4 * R])
                nc.vector.scalar_tensor_tensor(
                    W[:, R:3 * R], TC[:], np.pi / 4, P[:, 4 * R:6 * R],
                    op0=Alu.add, op1=Alu.mult)
                U = ps.tile([128, 3 * R], dt, tag="u")
                nc.vector.tensor_sub(U[:, 0:R], W[:, 0:R], P[:, 0:R])
                nc.vector.tensor_sub(U[:, R:3 * R], W[:, R:3 * R], AV[:])
                Z = ps.tile([128, 3 * R], dt, tag="z")
                nc.gpsimd.tensor_mul(Z[:], U[:], U[:])
                ZC = ps.tile([128, 3 * R], dt, tag="zc")
                nc.vector.tensor_tensor(ZC[:], Z[:], P[:, 6 * R:9 * R],
                                        op=Alu.min)
                E = ps.tile([128, R], dt, tag="e")
                nc.gpsimd.tensor_add(E[:], ZC[:, 0:R], ZC[:, R:2 * R])
                nc.gpsimd.tensor_add(E[:], E[:], ZC[:, 2 * R:3 * R])
                nc.sync.dma_start(
                    O_t[b0:b0 + bc].rearrange("b c (k l) -> b c k l", k=KC),
                    E[:])
    return nc


def _get_program():
    if "nc" not in _PROGRAM_CACHE:
        nc = _build_program()
        nc.finalize()   # Bacc: register allocation / DCE / wait legalization
        _PROGRAM_CACHE["nc"] = nc
    return _PROGRAM_CACHE["nc"]


def _host_prep(atom_description, coords, mean, std, weight):
    ad = np.asarray(atom_description)
    coords = np.asarray(coords, dtype=np.float32)
    b, ch, rs, rn, an = (ad[:, i] for i in range(5))
    valid = (b >= 0) & (b < NB) & (ch >= 0) & (ch < MC) & (rs >= 0) & (rs < MR)

    def scat3(mask):
        A = np.full((NB, MC, MR, 3), PAD, np.float32)
        m = mask & valid
        A[b[m], ch[m], rs[m]] = coords[m]
        return A

    Narr, CAarr, Carr = scat3(an == 0), scat3(an == 1), scat3(an == 2)
    seq = np.full((NB, MC, MR), PAD_I, np.int64)
    m = (an == 1) & valid
    seq[b[m], ch[m], rs[m]] = rn[m]

    todo = ((Narr[:, :, 1:, 0] != PAD) & (Carr[:, :, :-1, 0] != PAD)
            & (CAarr[:, :, 1:, 0] != PAD) & (CAarr[:, :, :-1, 0] != PAD)
            & (seq[:, :, 1:] != PAD_I) & (seq[:, :, :-1] != PAD_I))
    sidx = np.clip(np.where(todo, seq[:, :, 1:], 0), 0, 19)

    w0 = float(np.asarray(weight).reshape(-1)[0])
    s_w = 1.0 - np.tanh(-w0)
    sq = np.sqrt(s_w)
    mu = np.asarray(mean, np.float64)
    sd = np.asarray(std, np.float64)
    q = 1.0 / (sd * np.sqrt(2.0))
    tab = np.empty((20, 9))
    tab[:, 0] = mu[:, 0] * q[:, 0] * sq
    tab[:, 1] = (np.pi / 2 - mu[:, 1]) * q[:, 1] * sq
    tab[:, 2] = (mu[:, 2] - np.pi / 2) * q[:, 2] * sq
    tab[:, 3:6] = q * sq
    tab[:, 6:9] = s_w * np.maximum(np.log(CL * q), 0.0)
    tab = tab.astype(np.float32)

    params = np.zeros((NB, MC, MR, 9), np.float32)
    params[:, :, 1:, :] = tab[sidx] * todo[..., None].astype(np.float32)
    # blocked coefficient-plane layout [NB, MC, KC, 9, R]
    pblk = np.ascontiguousarray(
        params.reshape(NB, MC, KC, R, 9).transpose(0, 1, 2, 4, 3))

    G = np.zeros((NB, MC, MR + 1, 9), np.float32)
    G[:, :, 1:, 0:3] = Narr
    G[:, :, 1:, 3:6] = CAarr
    G[:, :, 1:, 6:9] = Carr
    # blocked plane-contiguous with halo: GB[b,c,k,p,l] = G[b,c,k*R+l,p]
    GB = np.empty((NB, MC, KC, 9, R + 1), np.float32)
    for k in range(KC):
        GB[:, :, k] = G[:, :, k * R:k * R + R + 1, :].transpose(0, 1, 3, 2)
    return GB, pblk


def _install_ntff_hook():
    """The agent image's antenv lacks axon_hooks; synthesize it so
    trace=True can reach the terminal's NRT profiler (dev-only path)."""
    import sys, types
    if "antenv.axon_hooks" in sys.modules:
        return True
    try:
        import antenv
        mod = types.ModuleType("antenv.axon_hooks")
        mod._hook = None

        def set_axon_ntff_profile_hook(h):
            mod._hook = h

        def get_axon_ntff_profile_hook():
            return mod._hook

        mod.set_axon_ntff_profile_hook = set_axon_ntff_profile_hook
        mod.get_axon_ntff_profile_hook = get_axon_ntff_profile_hook
        sys.modules["antenv.axon_hooks"] = mod
        antenv.axon_hooks = mod
        from trn_agent_boot.trn_boot import _ntff_profile_via_ctypes
        mod._hook = _ntff_profile_via_ctypes("/opt/axon/libaxon_pjrt.so")
        return True
    except Exception as e:  # pragma: no cover - profiling is best-effort
        print(f"ntff hook install failed: {e}")
        return False


def kernel(**inputs):
    global LAST_RESULT
    from concourse.bass_utils import run_bass_kernel_spmd
    if TRACE:
        _install_ntff_hook()

    G, pblk = _host_prep(
        inputs["atom_description"], inputs["coords"],
        inputs["mean"], inputs["std"], inputs["weight"])

    nc = _get_program()
    in_maps = [
        {"g": np.ascontiguousarray(G[i * BPC:(i + 1) * BPC]),
         "pr": np.ascontiguousarray(pblk[i * BPC:(i + 1) * BPC])}
        for i in range(NCORES)
    ]
    res = run_bass_kernel_spmd(nc, in_maps, list(range(NCORES)), trace=TRACE)
    LAST_RESULT = res
    e = np.concatenate([res.results[i]["out"] for i in range(NCORES)], axis=0)
    e = e.reshape(NB, MC, MR)
    out = np.repeat(e[..., None], NALT, axis=-1)
    return np.ascontiguousarray(out.astype(np.float32))
